# revision 3
# baseline (speedup 1.0000x reference)
"""Trainium2 Bass kernel for nn_CR8_reg_cond_mul_6 (moe_routing), v2.

Data-parallel over batch across 8 NeuronCores. Per core: 16 batches x 2048
tokens of a fused 1x1-conv chain + argmax routing + conditional matmuls.

v2 strategy (vs v1's 4-way-balanced engine load at ~60% occupancy):
- L1 stays f32r 3-term hi/lo (host pre-splits x, so no on-chip split ops).
- L2/L3 run as full-fp32 matmuls (4 cy/row vs 3) which ELIMINATES the
  h1/x2 hi+lo split chains (2 Pool + 2 DVE ops per tile and their serial
  ACT->Pool->DVE hops). PE becomes the single dominant engine.
- 1024-wide supertiles: one ACT evac for h1/x2 across 2 psum banks,
  1024-wide allreduce/is_equal/tok-copy.
- px (x_real reduce) matmuls software-pipelined one supertile late so the
  argmax chain (cls->allred->onehot->g) never stalls the PE queue.
- PSUM plan (8 banks): y-pair(2) + ycls(1) + ra(2) + rb(2) + pm|px(1,
  mask rows 0:16 and x_real rows 16:32 share one bank).
"""

import numpy as np

import concourse.bass as bass
import concourse.bacc as bacc
import concourse.tile as tile
import concourse.mybir as mybir
import concourse.bass_isa as bass_isa
from concourse import bass_utils

F32 = mybir.dt.float32
F32R = mybir.dt.float32r
FP8 = mybir.dt.float8e4

N_CORES = 8
B_FULL = 128
BS = B_FULL // N_CORES          # 16 batches per core
C = 128
W = 2048
T = 512                          # psum tile (bank = 512 fp32)
ST = 1024                        # supertile (2 psum banks)
NST = W // ST                    # 2 supertiles per batch
CLASSES = 128
SLOPE = 0.01
LRELU = mybir.ActivationFunctionType.Lrelu


def _round_f32r(x):
    """Round fp32 array to 11 explicit mantissa bits (matches HW f32r)."""
    x = np.ascontiguousarray(np.asarray(x, np.float32))
    xi = x.view(np.uint32)
    shift = np.uint32(12)  # 23 - 11
    half = np.uint32(1 << 11)
    mask = np.uint32(0xFFFFFFFF) << shift
    out = ((xi + half) & mask).view(np.float32).copy()
    out[~np.isfinite(x)] = x[~np.isfinite(x)]
    return out


def _split_f32r(x):
    hi = _round_f32r(x)
    lo = _round_f32r(np.asarray(x, np.float32) - hi)
    return hi, lo


def prepare_consts(cl1_w, cl1_b, cl2_w, cl2_b, cl3_w, cl3_b,
                   reg1_w, reg1_b, w2, b2, w3, b3):
    import ml_dtypes
    c = {}
    hi, lo = _split_f32r(cl1_w.T)
    c["w1hi"], c["w1lo"] = hi, lo                    # [128,128] f32r bits
    c["w2f"] = np.ascontiguousarray(cl2_w.T.astype(np.float32))
    c["c3f"] = np.ascontiguousarray(cl3_w[:CLASSES].T.astype(np.float32))
    c["b1"] = cl1_b.astype(np.float32).reshape(128, 1)
    c["b2c"] = cl2_b.astype(np.float32).reshape(128, 1)
    c["b3c"] = cl3_b[:CLASSES].astype(np.float32).reshape(128, 1)
    # mask row / x_real ones: 16-slot column-selector tables [128, 256]
    wm = _round_f32r(cl3_w[CLASSES:CLASSES + 1].T)   # [128,1]
    wm16s = np.zeros((128, 512), np.float32)
    ones16s = np.zeros((128, 512), np.float32)
    for sl in range(16):
        wm16s[:, 32 * sl + sl] = wm[:, 0]            # -> pm row sl
        ones16s[:, 32 * sl + 16 + sl] = 1.0 / CLASSES  # -> px row 16+sl
    c["wm16s"] = wm16s
    c["ones16s"] = ones16s
    c["bm16"] = np.full((16, 1), cl3_b[CLASSES], np.float32)
    c["wr"] = _round_f32r(reg1_w.T)
    c["br"] = reg1_b.astype(np.float32).reshape(128, 1)
    # CondMul1 fp8 DoubleRow tables: lhsT[kp, i, m] = W2all[kp + 128*i, m]
    w2all = np.transpose(w2, (1, 0, 2)).reshape(256, 256).astype(np.float32)
    w2dr = np.stack([w2all[0:128], w2all[128:256]], axis=1)      # [128,2,256]
    c["w2dra"] = w2dr[:, :, 0:128].astype(ml_dtypes.float8_e4m3)
    c["w2drb"] = w2dr[:, :, 128:256].astype(ml_dtypes.float8_e4m3)
    b2all = b2.reshape(256).astype(np.float32)
    c["b2a"] = b2all[0:128].reshape(128, 1)
    c["b2b"] = b2all[128:256].reshape(128, 1)
    # CondMul2 block table (unscaled; /128 lives in ones16s)
    w3x = np.zeros((256, CLASSES), np.float32)
    for cc in range(CLASSES):
        sc = cc // 16
        w3x[sc * 32:(sc + 1) * 32, cc] = w3[cc, :, 0]
    c["w3xdr"] = np.stack([w3x[0:128], w3x[128:256]], axis=1).astype(ml_dtypes.float8_e4m3)
    c["b3iota"] = (b3[:, 0].astype(np.float64)
                   + np.arange(CLASSES)).astype(np.float32).reshape(128, 1)
    return c


CONST_SPECS = [
    # ordered by first use; "q" = DMA issue queue (gp=SWDGE, sp=SP, act=ACT)
    ("w1hi", [128, 128], "f32", "gp"), ("w1lo", [128, 128], "f32", "gp"),
    ("wr", [128, 128], "f32", "gp"),
    ("b1", [128, 1], "f32", "sp"), ("br", [128, 1], "f32", "sp"),
    ("w2f", [128, 128], "f32", "gp"), ("b2c", [128, 1], "f32", "sp"),
    ("c3f", [128, 128], "f32", "gp"), ("b3c", [128, 1], "f32", "sp"),
    ("wm16s", [128, 512], "f32", "gp"), ("ones16s", [128, 512], "f32", "gp"),
    ("w2dra", [128, 2, 128], "fp8", "gp"), ("w2drb", [128, 2, 128], "fp8", "gp"),
    ("b2a", [128, 1], "f32", "sp"), ("b2b", [128, 1], "f32", "sp"),
    ("w3xdr", [128, 2, 128], "fp8", "gp"),
    ("b3iota", [128, 1], "f32", "sp"), ("bm16", [16, 1], "f32", "sp"),
]


def build_nc(bs=BS):
    nc = bacc.Bacc("TRN2", target_bir_lowering=False, debug=False)

    xhi_d = nc.dram_tensor("xhi", [bs, C, 1, W], F32, kind="ExternalInput")
    xlo_d = nc.dram_tensor("xlo", [bs, C, 1, W], F32, kind="ExternalInput")
    const_d = {}
    for name, shape, knd, _q in CONST_SPECS:
        dt = FP8 if knd == "fp8" else F32
        const_d[name] = nc.dram_tensor(name, shape, dt, kind="ExternalInput")
    xr_d = nc.dram_tensor("x_real", [bs, 1, 1, W], F32, kind="ExternalOutput")
    mk_d = nc.dram_tensor("mask", [bs, 1, 1, W], F32, kind="ExternalOutput")

    with tile.TileContext(nc) as tc:
        with (
            tc.tile_pool(name="consts", bufs=1) as cp,
            tc.tile_pool(name="io", bufs=6) as io,
            tc.tile_pool(name="acts", bufs=4) as ap,
            tc.tile_pool(name="sel", bufs=4) as sp,
            tc.tile_pool(name="gp", bufs=4) as gp,
            tc.tile_pool(name="outs", bufs=2) as op_,
            tc.tile_pool(name="py", bufs=3, space="PSUM") as py,      # y1 halves
            tc.tile_pool(name="py2", bufs=2, space="PSUM") as py2,    # y2/ycls halves
            tc.tile_pool(name="pra", bufs=2, space="PSUM") as pra,    # all reg-path
            tc.tile_pool(name="pmx", bufs=1, space="PSUM") as pmx,    # pm|px [32,512]
        ):
            # ---- load constants; f32r consts land in F32R tiles (host
            # pre-rounded, DMA is bitwise) so the BIR verifier sees f32r
            F32R_CONSTS = {"w1hi", "w1lo", "wr", "wm16s", "ones16s"}
            QUEUES = {"gp": nc.gpsimd, "sp": nc.sync, "act": nc.scalar, "spnow": nc.sync}
            cst = {}
            deferred_consts = []   # SP-queue consts issue after stream-0 DMAs
            for name, shape, knd, q in CONST_SPECS:
                dt = FP8 if knd == "fp8" else (F32R if name in F32R_CONSTS else F32)
                t = cp.tile(shape, dt, tag=f"c_{name}", name=f"c_{name}")
                src_ap = const_d[name].ap()
                if name in F32R_CONSTS:
                    src_ap = src_ap.bitcast(F32R)
                if q == "sp":
                    deferred_consts.append((t, src_ap))
                    cst[name] = t
                    continue
                else:
                    QUEUES[q].dma_start(t[:], src_ap)
                cst[name] = t

            w1hi = cst["w1hi"][:]
            w1lo = cst["w1lo"][:]
            wr = cst["wr"][:]
            wm16s = cst["wm16s"][:]
            ones16s = cst["ones16s"][:]

            xhv = xhi_d.ap().squeeze(2).bitcast(F32R)
            xlv = xlo_d.ap().squeeze(2).bitcast(F32R)
            assert bs % 4 == 0
            xrv = (xr_d.ap().squeeze(2).squeeze(1)
                   .rearrange("(g four) (n t) -> g (four n) t", four=4, t=T))
            mkv = (mk_d.ap().squeeze(2).squeeze(1)
                   .rearrange("(g four) (n t) -> g (four n) t", four=4, t=T))

            NSTEPS = bs * NST           # 32 supertiles
            STEPS_PER_GROUP = 4 * NST   # 8

            # carried state for the one-supertile-deferred px stage
            prev = None   # (g_ap, slots, pm_px_ap, group_idx, is_group_last)
            pmpx = None   # current group's [32, T] psum tile
            xr_pend = None  # (pmpx_ap, group) awaiting xr-evac after px flush

            def emit_px(state):
                """Emit the deferred px matmuls for supertile state; returns
                xr-evac closure if that supertile closed its group."""
                g_ap, slots, pm_px, grp, last = state
                for h in (0, 1):
                    s = slots[h]
                    nc.tensor.matmul(pm_px[0:32, :],
                                     ones16s[:, 32 * s:32 * s + 32],
                                     g_ap[:, h * T:(h + 1) * T],
                                     start=False, stop=(s == 15),
                                     skip_group_check=True)
                return (pm_px, grp) if last else None

            def emit_xr_evac(pend):
                pm_px, grp = pend
                xr_sb = op_.tile([32, T], F32, tag="xr", name="xr_sb")
                nc.vector.tensor_copy(xr_sb[:], pm_px[0:32, :])
                nc.sync.dma_start(xrv[grp], xr_sb[16:32, :])

            for n in range(NSTEPS):
                b, sti = divmod(n, NST)
                grp = b // 4
                group_first = (n % STEPS_PER_GROUP == 0)
                group_last = (n % STEPS_PER_GROUP == STEPS_PER_GROUP - 1)
                slots = [(b % 4) * 4 + sti * 2 + h for h in (0, 1)]

                # ---- DMA in (SP queue)
                xhi = io.tile([128, ST], F32R, tag="xhi", name="xhi_t")
                xlo = io.tile([128, ST], F32R, tag="xlo", name="xlo_t")
                if n == 0:
                    for h in (0, 1):
                        sl = slice(h * T, (h + 1) * T)
                        nc.sync.dma_start(xhi[:, sl], xhv[b, :, sti * ST + h * T:
                                                          sti * ST + (h + 1) * T])
                    for h in (0, 1):
                        sl = slice(h * T, (h + 1) * T)
                        nc.sync.dma_start(xlo[:, sl], xlv[b, :, sti * ST + h * T:
                                                          sti * ST + (h + 1) * T])
                else:
                    nc.sync.dma_start(xhi[:], xhv[b, :, sti * ST:(sti + 1) * ST])
                    nc.sync.dma_start(xlo[:], xlv[b, :, sti * ST:(sti + 1) * ST])
                if n == 1:
                    for t, src_ap in deferred_consts:
                        nc.sync.dma_start(t[:], src_ap)

                # ---- PE: L1 3-term f32r, per-half psum banks (tag rot)
                y1 = []
                for h in (0, 1):
                    t = py.tile([128, T], F32, tag="y1", name="y1h")
                    y1.append(t)
                for h in (0, 1):
                    sl = slice(h * T, (h + 1) * T)
                    nc.tensor.matmul(y1[h][:], w1hi, xhi[:, sl], start=True, stop=False)
                    nc.tensor.matmul(y1[h][:], w1hi, xlo[:, sl], start=False, stop=False)
                for h in (0, 1):
                    sl = slice(h * T, (h + 1) * T)
                    nc.tensor.matmul(y1[h][:], w1lo, xhi[:, sl], start=False, stop=True)

                # ---- PE: reg1 (independent early PE work)
                prs = []
                for h in (0, 1):
                    pr = pra.tile([128, T], F32, tag="ra", name="pr")
                    nc.tensor.matmul(pr[:], wr, xhi[:, h * T:(h + 1) * T])
                    prs.append(pr)

                # ---- PE: deferred px of previous supertile
                if prev is not None:
                    xr_pend = emit_px(prev) or xr_pend

                # ---- ACT: h1 = lrelu(y1 + b1) per half; DVE: tok fp8 copy
                h1f = ap.tile([128, ST], F32, tag="h1f", name="h1f")
                tok = ap.tile([128, 2 * ST], FP8, tag="tok", name="tok")
                for h in (0, 1):
                    sl = slice(h * T, (h + 1) * T)
                    nc.scalar.activation(h1f[:, sl], y1[h][:], LRELU,
                                         bias=cst["b1"][:], scale=1.0, alpha=SLOPE)
                    nc.vector.tensor_copy(tok[:, ST + h * T:ST + (h + 1) * T],
                                          h1f[:, sl])
                    # ACT: tok half h = lrelu(pr + br) fp8
                    nc.scalar.activation(tok[:, sl], prs[h][:], LRELU,
                                         bias=cst["br"][:], scale=1.0, alpha=SLOPE)

                # ---- PE: L2 full fp32 per half; ACT: x2 evac per half
                y2 = []
                x2f = ap.tile([128, ST], F32, tag="x2f", name="x2f")
                for h in (0, 1):
                    t = py2.tile([128, T], F32, tag="y2", name="y2h")
                    nc.tensor.matmul(t[:], cst["w2f"][:], h1f[:, h * T:(h + 1) * T],
                                     start=True, stop=True)
                    y2.append(t)
                for h in (0, 1):
                    sl = slice(h * T, (h + 1) * T)
                    nc.scalar.activation(x2f[:, sl], y2[h][:], LRELU,
                                         bias=cst["b2c"][:], scale=1.0, alpha=SLOPE)

                if group_first:
                    pmpx = pmx.tile([32, T], F32, tag="pmpx", name="pmpx")

                # ---- PE: L3 fp32 into y2-tag rotation (per-half natural deps)
                cls = sp.tile([128, ST], F32, tag="cls", name="cls_sb")
                maxbc = sp.tile([128, ST], F32, tag="maxbc", name="maxbc")
                onehot = sp.tile([128, ST], F32R, tag="onehot", name="onehot")
                for h in (0, 1):
                    sl = slice(h * T, (h + 1) * T)
                    ycls = py2.tile([128, T], F32, tag="y2", name="ycls")
                    nc.tensor.matmul(ycls[:], cst["c3f"][:], x2f[:, sl],
                                     start=True, stop=True)
                    s = slots[h]
                    x2r = sp.tile([128, T], F32R, tag="x2r", name="x2r")
                    nc.gpsimd.tensor_copy(x2r[:], x2f[:, sl])
                    nc.tensor.matmul(pmpx[0:32, :], wm16s[:, 32 * s:32 * s + 32],
                                     x2r[:],
                                     start=(s == 0), stop=False,
                                     skip_group_check=True)
                    # DVE: cls evac with exact fp32 bias add
                    nc.vector.tensor_scalar(out=cls[:, sl], in0=ycls[:],
                                            scalar1=cst["b3c"][:], scalar2=None,
                                            op0=mybir.AluOpType.add)
                    # Pool/DVE: per-half argmax allreduce + onehot
                    nc.gpsimd.partition_all_reduce(maxbc[:, sl], cls[:, sl],
                                                   channels=128,
                                                   reduce_op=bass_isa.ReduceOp.max)
                    nc.vector.tensor_tensor(onehot[:, sl], cls[:, sl], maxbc[:, sl],
                                            op=mybir.AluOpType.is_equal)

                # ---- PE/ACT: CondMul1 a-half (fp8 DoubleRow) into ra rotation
                tok3 = tok[:].rearrange("p (two x) -> p two x", two=2)
                hdr = ap.tile([128, 2 * ST], FP8, tag="hdr", name="hdr")
                for h in (0, 1):
                    sl = slice(h * T, (h + 1) * T)
                    pha = pra.tile([128, T], F32, tag="ra", name="pha")
                    nc.tensor.matmul(pha[:], cst["w2dra"][:],
                                     tok3[:, :, h * T:(h + 1) * T],
                                     perf_mode=mybir.MatmulPerfMode.DoubleRow)
                    nc.scalar.activation(hdr[:, sl], pha[:], LRELU,
                                         bias=cst["b2a"][:], scale=1.0, alpha=SLOPE)

                # ---- PE: CondMul1 b-half + CondMul2 + DVE g, per half
                hdr3 = hdr[:].rearrange("p (two x) -> p two x", two=2)
                g_t = gp.tile([128, ST], F32R, tag="g", name="g_t")
                for h in (0, 1):
                    sl = slice(h * T, (h + 1) * T)
                    phb = pra.tile([128, T], F32, tag="ra", name="phb")
                    nc.tensor.matmul(phb[:], cst["w2drb"][:],
                                     tok3[:, :, h * T:(h + 1) * T],
                                     perf_mode=mybir.MatmulPerfMode.DoubleRow)
                    nc.scalar.activation(hdr[:, ST + h * T:ST + (h + 1) * T],
                                         phb[:], LRELU,
                                         bias=cst["b2b"][:], scale=1.0, alpha=SLOPE)
                    preg = pra.tile([128, T], F32, tag="ra", name="preg")
                    nc.tensor.matmul(preg[:], cst["w3xdr"][:],
                                     hdr3[:, :, h * T:(h + 1) * T],
                                     perf_mode=mybir.MatmulPerfMode.DoubleRow)
                    nc.vector.scalar_tensor_tensor(
                        g_t[:, sl], in0=preg[:],
                        scalar=cst["b3iota"][:],
                        in1=onehot[:, sl].bitcast(F32),
                        op0=mybir.AluOpType.add, op1=mybir.AluOpType.mult)

                # ---- group-last: mask evac + out DMA (ACT + SP)
                if group_last:
                    mk_sb = op_.tile([16, T], F32, tag="mk", name="mk_sb")
                    nc.scalar.activation(mk_sb[:], pmpx[0:16, :], LRELU,
                                         bias=cst["bm16"][:], scale=1.0, alpha=SLOPE)
                    nc.sync.dma_start(mkv[grp], mk_sb[:])

                # ---- deferred xr evac from previous group (after its px flush)
                if xr_pend is not None and prev is not None:
                    emit_xr_evac(xr_pend)
                    xr_pend = None

                prev = (g_t[:], slots, pmpx[:], grp, group_last)

            # flush: px of the last supertile + final xr evac
            xr_pend = emit_px(prev) or xr_pend
            if xr_pend is not None:
                emit_xr_evac(xr_pend)

    nc.compile()
    return nc


_CACHE = {}


def kernel(x_in, cl1_w, cl1_b, cl2_w, cl2_b, cl3_w, cl3_b,
           reg1_w, reg1_b, w2, b2, w3, b3):
    if "nc" not in _CACHE:
        _CACHE["nc"] = build_nc()
    nc = _CACHE["nc"]

    consts = prepare_consts(cl1_w, cl1_b, cl2_w, cl2_b, cl3_w, cl3_b,
                            reg1_w, reg1_b, w2, b2, w3, b3)
    x_in = np.ascontiguousarray(np.asarray(x_in, np.float32))
    xhi = _round_f32r(x_in)
    xlo = _round_f32r(x_in - xhi)
    in_maps = []
    for core in range(N_CORES):
        sl = slice(core * BS, (core + 1) * BS)
        m = {"xhi": np.ascontiguousarray(xhi[sl]),
             "xlo": np.ascontiguousarray(xlo[sl])}
        m.update(consts)
        in_maps.append(m)

    res = bass_utils.run_bass_kernel_spmd(nc, in_maps, core_ids=list(range(N_CORES)))
    x_real = np.concatenate([r["x_real"] for r in res.results], axis=0)
    mask = np.concatenate([r["mask"] for r in res.results], axis=0)
    return x_real, mask


# revision 5
# speedup vs baseline: 1.0062x; 1.0062x over previous
"""Trainium2 Bass kernel for nn_CR8_reg_cond_mul_6 (moe_routing), v2.

Data-parallel over batch across 8 NeuronCores. Per core: 16 batches x 2048
tokens of a fused 1x1-conv chain + argmax routing + conditional matmuls.

v2 strategy (vs v1's 4-way-balanced engine load at ~60% occupancy):
- L1 stays f32r 3-term hi/lo (host pre-splits x, so no on-chip split ops).
- L2/L3 run as full-fp32 matmuls (4 cy/row vs 3) which ELIMINATES the
  h1/x2 hi+lo split chains (2 Pool + 2 DVE ops per tile and their serial
  ACT->Pool->DVE hops). PE becomes the single dominant engine.
- 1024-wide supertiles: one ACT evac for h1/x2 across 2 psum banks,
  1024-wide allreduce/is_equal/tok-copy.
- px (x_real reduce) matmuls software-pipelined one supertile late so the
  argmax chain (cls->allred->onehot->g) never stalls the PE queue.
- PSUM plan (8 banks): y-pair(2) + ycls(1) + ra(2) + rb(2) + pm|px(1,
  mask rows 0:16 and x_real rows 16:32 share one bank).
"""

import numpy as np

import concourse.bass as bass
import concourse.bacc as bacc
import concourse.tile as tile
import concourse.mybir as mybir
import concourse.bass_isa as bass_isa
from concourse import bass_utils

F32 = mybir.dt.float32
F32R = mybir.dt.float32r
FP8 = mybir.dt.float8e4

N_CORES = 8
B_FULL = 128
BS = B_FULL // N_CORES          # 16 batches per core
C = 128
W = 2048
T = 512                          # psum tile (bank = 512 fp32)
ST = 1024                        # supertile (2 psum banks)
NST = W // ST                    # 2 supertiles per batch
CLASSES = 128
SLOPE = 0.01
LRELU = mybir.ActivationFunctionType.Lrelu


def _round_f32r(x):
    """Round fp32 array to 11 explicit mantissa bits (matches HW f32r)."""
    x = np.ascontiguousarray(np.asarray(x, np.float32))
    xi = x.view(np.uint32)
    shift = np.uint32(12)  # 23 - 11
    half = np.uint32(1 << 11)
    mask = np.uint32(0xFFFFFFFF) << shift
    out = ((xi + half) & mask).view(np.float32).copy()
    out[~np.isfinite(x)] = x[~np.isfinite(x)]
    return out


def _split_f32r(x):
    hi = _round_f32r(x)
    lo = _round_f32r(np.asarray(x, np.float32) - hi)
    return hi, lo


def prepare_consts(cl1_w, cl1_b, cl2_w, cl2_b, cl3_w, cl3_b,
                   reg1_w, reg1_b, w2, b2, w3, b3):
    import ml_dtypes
    c = {}
    hi, lo = _split_f32r(cl1_w.T)
    c["w1hi"], c["w1lo"] = hi, lo                    # [128,128] f32r bits
    c["w2f"] = np.ascontiguousarray(cl2_w.T.astype(np.float32))
    c["c3f"] = np.ascontiguousarray(cl3_w[:CLASSES].T.astype(np.float32))
    c["b1"] = cl1_b.astype(np.float32).reshape(128, 1)
    c["b2c"] = cl2_b.astype(np.float32).reshape(128, 1)
    c["b3c"] = cl3_b[:CLASSES].astype(np.float32).reshape(128, 1)
    # mask row / x_real ones: 16-slot column-selector tables [128, 256]
    wm = _round_f32r(cl3_w[CLASSES:CLASSES + 1].T)   # [128,1]
    wm16s = np.zeros((128, 512), np.float32)
    ones16s = np.zeros((128, 512), np.float32)
    for sl in range(16):
        wm16s[:, 32 * sl + sl] = wm[:, 0]            # -> pm row sl
        ones16s[:, 32 * sl + 16 + sl] = 1.0 / CLASSES  # -> px row 16+sl
    c["wm16s"] = wm16s
    c["ones16s"] = ones16s
    c["bm16"] = np.full((16, 1), cl3_b[CLASSES], np.float32)
    c["wr"] = _round_f32r(reg1_w.T)
    c["br"] = reg1_b.astype(np.float32).reshape(128, 1)
    # CondMul1 fp8 DoubleRow tables: lhsT[kp, i, m] = W2all[kp + 128*i, m]
    w2all = np.transpose(w2, (1, 0, 2)).reshape(256, 256).astype(np.float32)
    w2dr = np.stack([w2all[0:128], w2all[128:256]], axis=1)      # [128,2,256]
    c["w2dra"] = w2dr[:, :, 0:128].astype(ml_dtypes.float8_e4m3)
    c["w2drb"] = w2dr[:, :, 128:256].astype(ml_dtypes.float8_e4m3)
    b2all = b2.reshape(256).astype(np.float32)
    c["b2a"] = b2all[0:128].reshape(128, 1)
    c["b2b"] = b2all[128:256].reshape(128, 1)
    # CondMul2 block table (unscaled; /128 lives in ones16s)
    w3x = np.zeros((256, CLASSES), np.float32)
    for cc in range(CLASSES):
        sc = cc // 16
        w3x[sc * 32:(sc + 1) * 32, cc] = w3[cc, :, 0]
    c["w3xdr"] = np.stack([w3x[0:128], w3x[128:256]], axis=1).astype(ml_dtypes.float8_e4m3)
    c["b3iota"] = (b3[:, 0].astype(np.float64)
                   + np.arange(CLASSES)).astype(np.float32).reshape(128, 1)
    return c


CONST_SPECS = [
    # ordered by first use; "q" = DMA issue queue (gp=SWDGE, sp=SP, act=ACT)
    ("w1hi", [128, 128], "f32", "gp"), ("w1lo", [128, 128], "f32", "gp"),
    ("wr", [128, 128], "f32", "gp"),
    ("b1", [128, 1], "f32", "sp"), ("br", [128, 1], "f32", "sp"),
    ("w2f", [128, 128], "f32", "gp"), ("b2c", [128, 1], "f32", "sp"),
    ("c3f", [128, 128], "f32", "gp"), ("b3c", [128, 1], "f32", "sp"),
    ("wm16s", [128, 512], "f32", "gp"), ("ones16s", [128, 512], "f32", "gp"),
    ("w2dra", [128, 2, 128], "fp8", "gp"), ("w2drb", [128, 2, 128], "fp8", "gp"),
    ("b2a", [128, 1], "f32", "sp"), ("b2b", [128, 1], "f32", "sp"),
    ("w3xdr", [128, 2, 128], "fp8", "gp"),
    ("b3iota", [128, 1], "f32", "sp"), ("bm16", [16, 1], "f32", "sp"),
]


def build_nc(bs=BS):
    nc = bacc.Bacc("TRN2", target_bir_lowering=False, debug=False)

    xhi_d = nc.dram_tensor("xhi", [bs, C, 1, W], F32, kind="ExternalInput")
    xlo_d = nc.dram_tensor("xlo", [bs, C, 1, W], F32, kind="ExternalInput")
    const_d = {}
    for name, shape, knd, _q in CONST_SPECS:
        dt = FP8 if knd == "fp8" else F32
        const_d[name] = nc.dram_tensor(name, shape, dt, kind="ExternalInput")
    xr_d = nc.dram_tensor("x_real", [bs, 1, 1, W], F32, kind="ExternalOutput")
    mk_d = nc.dram_tensor("mask", [bs, 1, 1, W], F32, kind="ExternalOutput")

    with tile.TileContext(nc) as tc:
        with (
            tc.tile_pool(name="consts", bufs=1) as cp,
            tc.tile_pool(name="io", bufs=6) as io,
            tc.tile_pool(name="acts", bufs=4) as ap,
            tc.tile_pool(name="sel", bufs=4) as sp,
            tc.tile_pool(name="gp", bufs=4) as gp,
            tc.tile_pool(name="outs", bufs=2) as op_,
            tc.tile_pool(name="py", bufs=3, space="PSUM") as py,      # y1 halves
            tc.tile_pool(name="py2", bufs=2, space="PSUM") as py2,    # y2/ycls halves
            tc.tile_pool(name="pra", bufs=2, space="PSUM") as pra,    # all reg-path
            tc.tile_pool(name="pmx", bufs=1, space="PSUM") as pmx,    # pm|px [32,512]
        ):
            # ---- load constants; f32r consts land in F32R tiles (host
            # pre-rounded, DMA is bitwise) so the BIR verifier sees f32r
            F32R_CONSTS = {"w1hi", "w1lo", "wr", "wm16s", "ones16s"}
            QUEUES = {"gp": nc.gpsimd, "sp": nc.sync, "act": nc.scalar, "spnow": nc.sync}
            cst = {}
            deferred_consts = []   # SP-queue consts issue after stream-0 DMAs
            for name, shape, knd, q in CONST_SPECS:
                dt = FP8 if knd == "fp8" else (F32R if name in F32R_CONSTS else F32)
                t = cp.tile(shape, dt, tag=f"c_{name}", name=f"c_{name}")
                src_ap = const_d[name].ap()
                if name in F32R_CONSTS:
                    src_ap = src_ap.bitcast(F32R)
                if q == "sp":
                    deferred_consts.append((t, src_ap))
                    cst[name] = t
                    continue
                else:
                    QUEUES[q].dma_start(t[:], src_ap)
                cst[name] = t

            w1hi = cst["w1hi"][:]
            w1lo = cst["w1lo"][:]
            wr = cst["wr"][:]
            wm16s = cst["wm16s"][:]
            ones16s = cst["ones16s"][:]

            xhv = xhi_d.ap().squeeze(2).bitcast(F32R)
            xlv = xlo_d.ap().squeeze(2).bitcast(F32R)
            assert bs % 4 == 0
            xrv = (xr_d.ap().squeeze(2).squeeze(1)
                   .rearrange("(g four) (n t) -> g (four n) t", four=4, t=T))
            mkv = (mk_d.ap().squeeze(2).squeeze(1)
                   .rearrange("(g four) (n t) -> g (four n) t", four=4, t=T))

            PRIO_EVAC = 40
            NSTEPS = bs * NST           # 32 supertiles
            STEPS_PER_GROUP = 4 * NST   # 8

            # carried state for the one-supertile-deferred px stage
            prev = None   # (g_ap, slots, pm_px_ap, group_idx, is_group_last)
            pmpx = None   # current group's [32, T] psum tile
            xr_pend = None  # (pmpx_ap, group) awaiting xr-evac after px flush

            def emit_px(state):
                """Emit the deferred mask+px matmuls for supertile state;
                returns xr-evac closure if that supertile closed its group."""
                g_ap, slots, pm_px, grp, last, x2rs = state
                for h in (0, 1):
                    s = slots[h]
                    nc.tensor.matmul(pm_px[0:32, :],
                                     wm16s[:, 32 * s:32 * s + 32],
                                     x2rs[h],
                                     start=(s == 0), stop=False,
                                     skip_group_check=True)
                    nc.tensor.matmul(pm_px[0:32, :],
                                     ones16s[:, 32 * s:32 * s + 32],
                                     g_ap[:, h * T:(h + 1) * T],
                                     start=False, stop=(s == 15),
                                     skip_group_check=True)
                if not last:
                    return None
                # group complete: mask evac (all 16 pm slots now written)
                mk_sb = op_.tile([16, T], F32, tag="mk", name="mk_sb")
                nc.scalar.activation(mk_sb[:], pm_px[0:16, :], LRELU,
                                     bias=cst["bm16"][:], scale=1.0, alpha=SLOPE)
                nc.sync.dma_start(mkv[grp], mk_sb[:])
                return (pm_px, grp)

            def emit_xr_evac(pend):
                pm_px, grp = pend
                xr_sb = op_.tile([32, T], F32, tag="xr", name="xr_sb")
                nc.vector.tensor_copy(xr_sb[:], pm_px[0:32, :])
                nc.sync.dma_start(xrv[grp], xr_sb[16:32, :])

            for n in range(NSTEPS):
                b, sti = divmod(n, NST)
                grp = b // 4
                group_first = (n % STEPS_PER_GROUP == 0)
                group_last = (n % STEPS_PER_GROUP == STEPS_PER_GROUP - 1)
                slots = [(b % 4) * 4 + sti * 2 + h for h in (0, 1)]

                # ---- DMA in (SP queue)
                xhi = io.tile([128, ST], F32R, tag="xhi", name="xhi_t")
                xlo = io.tile([128, ST], F32R, tag="xlo", name="xlo_t")
                if n == 0:
                    for h in (0, 1):
                        sl = slice(h * T, (h + 1) * T)
                        nc.sync.dma_start(xhi[:, sl], xhv[b, :, sti * ST + h * T:
                                                          sti * ST + (h + 1) * T])
                    for h in (0, 1):
                        sl = slice(h * T, (h + 1) * T)
                        nc.sync.dma_start(xlo[:, sl], xlv[b, :, sti * ST + h * T:
                                                          sti * ST + (h + 1) * T])
                else:
                    nc.sync.dma_start(xhi[:], xhv[b, :, sti * ST:(sti + 1) * ST])
                    nc.sync.dma_start(xlo[:], xlv[b, :, sti * ST:(sti + 1) * ST])
                if n == 1:
                    for t, src_ap in deferred_consts:
                        nc.sync.dma_start(t[:], src_ap)

                # ---- PE: L1 3-term f32r, per-half psum banks (tag rot)
                y1 = []
                for h in (0, 1):
                    t = py.tile([128, T], F32, tag="y1", name="y1h")
                    y1.append(t)
                for h in (0, 1):
                    sl = slice(h * T, (h + 1) * T)
                    nc.tensor.matmul(y1[h][:], w1hi, xhi[:, sl], start=True, stop=False)
                    nc.tensor.matmul(y1[h][:], w1hi, xlo[:, sl], start=False, stop=False)
                for h in (0, 1):
                    sl = slice(h * T, (h + 1) * T)
                    nc.tensor.matmul(y1[h][:], w1lo, xhi[:, sl], start=False, stop=True)

                # ---- PE: reg1 (independent early PE work)
                prs = []
                for h in (0, 1):
                    pr = pra.tile([128, T], F32, tag="ra", name="pr")
                    nc.tensor.matmul(pr[:], wr, xhi[:, h * T:(h + 1) * T])
                    prs.append(pr)

                # ---- PE: deferred px of previous supertile
                if prev is not None:
                    xr_pend = emit_px(prev) or xr_pend

                # ---- ACT: h1 = lrelu(y1 + b1) per half; DVE: tok fp8 copy
                h1f = ap.tile([128, ST], F32, tag="h1f", name="h1f")
                tok = ap.tile([128, 2 * ST], FP8, tag="tok", name="tok")
                for h in (0, 1):
                    sl = slice(h * T, (h + 1) * T)
                    with tc.high_priority(offset=PRIO_EVAC):
                        nc.scalar.activation(h1f[:, sl], y1[h][:], LRELU,
                                             bias=cst["b1"][:], scale=1.0, alpha=SLOPE)
                    with tc.high_priority(offset=PRIO_EVAC):
                        nc.vector.tensor_copy(tok[:, ST + h * T:ST + (h + 1) * T],
                                              h1f[:, sl])
                    # ACT: tok half h = lrelu(pr + br) fp8
                    nc.scalar.activation(tok[:, sl], prs[h][:], LRELU,
                                         bias=cst["br"][:], scale=1.0, alpha=SLOPE)

                # ---- PE: L2 full fp32 per half; ACT: x2 evac per half
                y2 = []
                x2f = ap.tile([128, ST], F32, tag="x2f", name="x2f")
                for h in (0, 1):
                    t = py2.tile([128, T], F32, tag="y2", name="y2h")
                    nc.tensor.matmul(t[:], cst["w2f"][:], h1f[:, h * T:(h + 1) * T],
                                     start=True, stop=True)
                    y2.append(t)
                for h in (0, 1):
                    sl = slice(h * T, (h + 1) * T)
                    with tc.high_priority(offset=PRIO_EVAC):
                        nc.scalar.activation(x2f[:, sl], y2[h][:], LRELU,
                                             bias=cst["b2c"][:], scale=1.0, alpha=SLOPE)

                if group_first:
                    pmpx = pmx.tile([32, T], F32, tag="pmpx", name="pmpx")

                # ---- PE: L3 fp32 into y2-tag rotation (per-half natural deps)
                cls = sp.tile([128, ST], F32, tag="cls", name="cls_sb")
                maxbc = sp.tile([128, ST], F32, tag="maxbc", name="maxbc")
                onehot = sp.tile([128, ST], F32R, tag="onehot", name="onehot")
                x2rs = []
                for h in (0, 1):
                    sl = slice(h * T, (h + 1) * T)
                    ycls = py2.tile([128, T], F32, tag="y2", name="ycls")
                    nc.tensor.matmul(ycls[:], cst["c3f"][:], x2f[:, sl],
                                     start=True, stop=True)
                    x2r = sp.tile([128, T], F32R, tag="x2r", name="x2r")
                    nc.gpsimd.tensor_copy(x2r[:], x2f[:, sl])
                    x2rs.append(x2r[:])
                    # DVE: cls evac with exact fp32 bias add
                    nc.vector.tensor_scalar(out=cls[:, sl], in0=ycls[:],
                                            scalar1=cst["b3c"][:], scalar2=None,
                                            op0=mybir.AluOpType.add)
                    # Pool/DVE: per-half argmax allreduce + onehot
                    nc.gpsimd.partition_all_reduce(maxbc[:, sl], cls[:, sl],
                                                   channels=128,
                                                   reduce_op=bass_isa.ReduceOp.max)
                    nc.vector.tensor_tensor(onehot[:, sl], cls[:, sl], maxbc[:, sl],
                                            op=mybir.AluOpType.is_equal)

                # ---- PE/ACT: CondMul1 a-half (fp8 DoubleRow) into ra rotation
                tok3 = tok[:].rearrange("p (two x) -> p two x", two=2)
                hdr = ap.tile([128, 2 * ST], FP8, tag="hdr", name="hdr")
                for h in (0, 1):
                    sl = slice(h * T, (h + 1) * T)
                    pha = pra.tile([128, T], F32, tag="ra", name="pha")
                    nc.tensor.matmul(pha[:], cst["w2dra"][:],
                                     tok3[:, :, h * T:(h + 1) * T],
                                     perf_mode=mybir.MatmulPerfMode.DoubleRow)
                    nc.scalar.activation(hdr[:, sl], pha[:], LRELU,
                                         bias=cst["b2a"][:], scale=1.0, alpha=SLOPE)

                # ---- PE: CondMul1 b-half + CondMul2 + DVE g, per half
                hdr3 = hdr[:].rearrange("p (two x) -> p two x", two=2)
                g_t = gp.tile([128, ST], F32R, tag="g", name="g_t")
                for h in (0, 1):
                    sl = slice(h * T, (h + 1) * T)
                    phb = pra.tile([128, T], F32, tag="ra", name="phb")
                    nc.tensor.matmul(phb[:], cst["w2drb"][:],
                                     tok3[:, :, h * T:(h + 1) * T],
                                     perf_mode=mybir.MatmulPerfMode.DoubleRow)
                    nc.scalar.activation(hdr[:, ST + h * T:ST + (h + 1) * T],
                                         phb[:], LRELU,
                                         bias=cst["b2b"][:], scale=1.0, alpha=SLOPE)
                    preg = pra.tile([128, T], F32, tag="ra", name="preg")
                    nc.tensor.matmul(preg[:], cst["w3xdr"][:],
                                     hdr3[:, :, h * T:(h + 1) * T],
                                     perf_mode=mybir.MatmulPerfMode.DoubleRow)
                    nc.vector.scalar_tensor_tensor(
                        g_t[:, sl], in0=preg[:],
                        scalar=cst["b3iota"][:],
                        in1=onehot[:, sl].bitcast(F32),
                        op0=mybir.AluOpType.add, op1=mybir.AluOpType.mult)


                # ---- deferred xr evac from previous group (after its px flush)
                if xr_pend is not None and prev is not None:
                    emit_xr_evac(xr_pend)
                    xr_pend = None

                prev = (g_t[:], slots, pmpx[:], grp, group_last, x2rs)

            # flush: px of the last supertile + final xr evac
            xr_pend = emit_px(prev) or xr_pend
            if xr_pend is not None:
                emit_xr_evac(xr_pend)

    nc.compile()
    return nc


_CACHE = {}


def kernel(x_in, cl1_w, cl1_b, cl2_w, cl2_b, cl3_w, cl3_b,
           reg1_w, reg1_b, w2, b2, w3, b3):
    if "nc" not in _CACHE:
        _CACHE["nc"] = build_nc()
    nc = _CACHE["nc"]

    consts = prepare_consts(cl1_w, cl1_b, cl2_w, cl2_b, cl3_w, cl3_b,
                            reg1_w, reg1_b, w2, b2, w3, b3)
    x_in = np.ascontiguousarray(np.asarray(x_in, np.float32))
    xhi = _round_f32r(x_in)
    xlo = _round_f32r(x_in - xhi)
    in_maps = []
    for core in range(N_CORES):
        sl = slice(core * BS, (core + 1) * BS)
        m = {"xhi": np.ascontiguousarray(xhi[sl]),
             "xlo": np.ascontiguousarray(xlo[sl])}
        m.update(consts)
        in_maps.append(m)

    res = bass_utils.run_bass_kernel_spmd(nc, in_maps, core_ids=list(range(N_CORES)))
    x_real = np.concatenate([r["x_real"] for r in res.results], axis=0)
    mask = np.concatenate([r["mask"] for r in res.results], axis=0)
    return x_real, mask
